# revision 4
# baseline (speedup 1.0000x reference)
"""Butterfly layer (12 stages over L=4096) on 8 Trainium2 NeuronCores.

Math: each stage s computes y[j] = W[s][j,0]*y[j] + W[s][j,1]*y[j^d], d=2^s.
The product of stages 0-6 is block-diagonal over contiguous 128-blocks of L
(32 dense 128x128 matrices C1).  Stages 7-11 mix positions {m*128+p : m} at
fixed p, i.e. a 32x32 dense matrix D_p per within-block position p; grouping
p by residue class mod 32 turns these into 32 block-diagonal 128x128
stationaries C2 after a 32x32 partition<->free exchange done on-chip by the
DVE stream transpose.

Device layout is transposed ([L on partitions, batch on free]); the host
transposes x / untransposes the result (free).  Data-parallel over batch
across the 8 cores.  Phase 1 runs in float32r (TF32-like, full PE speed at
N>=256), phase 2 in fp16.
"""
import numpy as np

BATCH, L, E = 8192, 4096, 12
N_CORES = 8
BCORE = BATCH // N_CORES  # 1024 batch columns per core
BC = 256                  # batch columns per superchunk
NSC = BCORE // BC         # 4 superchunks per core
NB = L // 128             # 32 partition blocks

_CACHE = {}


def _stage_coeffs(W):
    # W: (12, 4096, 2) float -> per-stage A (diag) and B (off-diag) in f64
    Wd = np.asarray(W, dtype=np.float64)
    return Wd[:, :, 0], Wd[:, :, 1]


def _build_c1(W):
    """C1[m*128 + k, mu] = (product of stages 0..6 on block m)[mu, k]."""
    A, B = _stage_coeffs(W)
    c1 = np.empty((L, 128), dtype=np.float64)
    for m in range(NB):
        Em = np.eye(128, dtype=np.float64)
        for s in range(7):
            d = 1 << s
            a = A[s, m * 128:(m + 1) * 128]
            b = B[s, m * 128:(m + 1) * 128]
            idx = np.arange(128) ^ d
            Em = a[:, None] * Em + b[:, None] * Em[idx, :]
        # out[mu] = sum_k Em[mu, k] y[k]  ->  lhsT[k, mu] = Em[mu, k]
        c1[m * 128:(m + 1) * 128, :] = Em.T
    return c1.astype(np.float32)


def _build_c2(W):
    """C2 for residue class c: rows k = 32q + m' (q = p//32 with p = 32q+c),
    cols mu = 4m + q'.  Entry = D_{32q+c}[m', m] when q==q', else 0, where
    D_p = F_p^T and F_p is the product of stages 7..11 on the coset
    {m*128+p : m}."""
    A, B = _stage_coeffs(W)
    c2 = np.zeros((L, 128), dtype=np.float64)
    idx32 = np.arange(32)
    for c in range(32):
        for q in range(4):
            p = 32 * q + c
            Fp = np.eye(32, dtype=np.float64)
            for s in range(7, 12):
                e = (1 << s) // 128
                a = A[s, idx32 * 128 + p]
                b = B[s, idx32 * 128 + p]
                Fp = a[:, None] * Fp + b[:, None] * Fp[idx32 ^ e, :]
            # out[m] = sum_{m'} Fp[m, m'] y1[m'] -> lhsT[m', m] = Fp[m, m']
            for mp in range(32):
                for m in range(32):
                    c2[c * 128 + 32 * q + mp, 4 * m + q] = Fp[m, mp]
    return c2.astype(np.float16)


def _split_excess_waits(nc):
    """The staged walrus rejects instructions carrying more than one sync-wait
    command.  Hoist all but the last semaphore wait of each instruction onto
    fresh same-engine nops placed immediately before it (engines execute
    their stream in order, so semantics are unchanged)."""
    from concourse import mybir

    snapshots = []
    for fn in nc.m.functions:
        for blk in fn.blocks:
            snapshots.append((blk, list(blk.instructions)))
    for blk, insts in snapshots:
        changed = False
        new_list = []
        for inst in insts:
            si = inst.sync_info
            if si is not None:
                waits = list(si.on_wait)
                sem_waits = [w for w in waits if w.sync_type == "semaphore"]
                other = [w for w in waits if w.sync_type != "semaphore"]
                budget = 1 if not other else 0
                if len(sem_waits) > budget:
                    keep = sem_waits[-budget:] if budget else []
                    hoist = sem_waits[: len(sem_waits) - budget]
                    for w in hoist:
                        nop = nc.engines[inst.engine].nop(
                            hint="waitsplit", nofuse=True
                        )
                        nop.ins.sync_info = mybir.SyncInfo(
                            on_wait=[w], on_update=[]
                        )
                        new_list.append(nop.ins)
                    si.on_wait = other + keep
                    changed = True
            new_list.append(inst)
        if changed:
            blk.instructions = new_list


def _build_program():
    import concourse.bass as bass
    import concourse.tile as tile
    from concourse import mybir

    f32 = mybir.dt.float32
    f32r = mybir.dt.float32r
    f16 = mybir.dt.float16

    nc = bass.Bass("TRN2", num_devices=N_CORES)
    xT = nc.dram_tensor("xT", [L, BCORE], f32, kind="ExternalInput").ap()
    c1 = nc.dram_tensor("c1", [L, 128], f32, kind="ExternalInput").ap()
    c2 = nc.dram_tensor("c2", [L, 128], f16, kind="ExternalInput").ap()
    outT = nc.dram_tensor("outT", [L, BCORE], f32, kind="ExternalOutput").ap()

    xT_r = xT.rearrange("(m p) b -> p m b", p=128)    # [128, 32, BCORE]
    c1_r = c1.rearrange("(m p) k -> p m k", p=128)    # [128, 32, 128]
    c2_r = c2.rearrange("(m p) k -> p m k", p=128)
    outT_r = outT.rearrange("(t c) b -> t c b", c=32)  # [128, 32, BCORE]

    with tile.TileContext(nc) as tc:
        with (
            tc.tile_pool(name="const", bufs=1) as cpool,
            tc.tile_pool(name="xin", bufs=2) as xpool,
            tc.tile_pool(name="y1", bufs=2) as y1pool,
            tc.tile_pool(name="y1t", bufs=2) as y1tpool,
            tc.tile_pool(name="yo", bufs=3) as yopool,
            tc.tile_pool(name="ps1", bufs=2, space="PSUM") as ps1pool,
            tc.tile_pool(name="ps2", bufs=2, space="PSUM") as ps2pool,
        ):
            c1t = cpool.tile([128, NB * 128], f32r, tag="c1t")
            nc.gpsimd.dma_start(
                c1t[:].rearrange("p (m k) -> p m k", m=NB), c1_r[:]
            )
            c2t = cpool.tile([128, NB * 128], f16, tag="c2t")
            nc.sync.dma_start(
                c2t[:].rearrange("p (m k) -> p m k", m=NB), c2_r[:]
            )

            for s in range(NSC):
                xin = xpool.tile([128, NB * BC], f32r, tag="xin")
                for h in range(4):
                    nc.gpsimd.dma_start(
                        xin[:, h * 8 * BC:(h + 1) * 8 * BC].rearrange(
                            "p (m b) -> p m b", m=8
                        ),
                        xT_r[:, h * 8:(h + 1) * 8, s * BC:(s + 1) * BC],
                    )
                y1 = y1pool.tile([128, NB * BC], f16, tag="y1")
                for g in range(8):
                    ps = ps1pool.tile([128, 4 * BC], f32, tag="ps1")
                    for i in range(4):
                        m = 4 * g + i
                        nc.tensor.matmul(
                            ps[:, i * BC:(i + 1) * BC],
                            c1t[:, m * 128:(m + 1) * 128],
                            xin[:, m * BC:(m + 1) * BC],
                            start=True,
                            stop=True,
                        )
                    nc.any.tensor_copy(
                        y1[:, g * 4 * BC:(g + 1) * 4 * BC], ps[:]
                    )
                # 32x32 partition<->free exchange: in stream (b outer, m
                # inner), out stream (b outer, c inner)
                y1t = y1tpool.tile([128, NB * BC], f16, tag="y1t")
                nc.vector.transpose(
                    y1t[:].rearrange("p (c b) -> p b c", c=NB),
                    y1[:].rearrange("p (m b) -> p b m", m=NB),
                )
                for g in range(8):
                    ps2 = ps2pool.tile([128, 4 * BC], f32, tag="ps2")
                    for i in range(4):
                        c = 4 * g + i
                        nc.tensor.matmul(
                            ps2[:, i * BC:(i + 1) * BC],
                            c2t[:, c * 128:(c + 1) * 128],
                            y1t[:, c * BC:(c + 1) * BC],
                            start=True,
                            stop=True,
                        )
                    yo = yopool.tile([128, 4 * BC], f32, tag="yo")
                    nc.any.tensor_copy(yo[:], ps2[:])
                    nc.sync.dma_start(
                        outT_r[:, 4 * g:4 * (g + 1), s * BC:(s + 1) * BC],
                        yo[:].rearrange("p (c b) -> p c b", c=4),
                    )
    _split_excess_waits(nc)
    return nc


def _get_program():
    if "nc" not in _CACHE:
        _CACHE["nc"] = _build_program()
    return _CACHE["nc"]


def kernel(x: np.ndarray, W: np.ndarray) -> np.ndarray:
    from concourse.bass_utils import run_bass_kernel_spmd

    x = np.ascontiguousarray(np.asarray(x, dtype=np.float32))
    c1 = _build_c1(W)
    c2 = _build_c2(W)
    xT = np.ascontiguousarray(x.T)  # [L, BATCH]

    nc = _get_program()
    in_maps = []
    for core in range(N_CORES):
        sl = slice(core * BCORE, (core + 1) * BCORE)
        in_maps.append(
            {
                "xT": np.ascontiguousarray(xT[:, sl]),
                "c1": c1,
                "c2": c2,
            }
        )
    _CACHE["in_maps"] = in_maps
    res = run_bass_kernel_spmd(nc, in_maps, list(range(N_CORES)))
    out = np.empty((BATCH, L), dtype=np.float32)
    for core in range(N_CORES):
        out[core * BCORE:(core + 1) * BCORE, :] = res.results[core]["outT"].T
    return out


# revision 23
# speedup vs baseline: 112.7072x; 112.7072x over previous
"""Butterfly layer (12 stages over L=4096) on 8 Trainium2 NeuronCores.

Math: each stage s computes y[j] = W[s][j,0]*y[j] + W[s][j,1]*y[j^d], d=2^s.
The product of stages 0-6 is block-diagonal over contiguous 128-blocks of L
(32 dense 128x128 matrices C1).  Stages 7-11 mix positions {m*128+p : m} at
fixed p, i.e. a 32x32 dense matrix D_p per within-block position p; grouping
p by residue class mod 32 turns these into 32 block-diagonal 128x128
stationaries C2 after a 32x32 partition<->free exchange done on-chip by the
DVE stream transpose.

Device layout is transposed ([L on partitions, batch on free]); the host
transposes x / untransposes the result (free).  Data-parallel over batch
across the 8 cores.  Phase 1 runs in float32r (TF32-like, full PE speed at
N>=256), phase 2 in fp16.
"""
import numpy as np

BATCH, L, E = 8192, 4096, 12
N_CORES = 8
BCORE = BATCH // N_CORES  # 1024 batch columns per core
BC = 256                  # batch columns per superchunk
NSC = BCORE // BC         # 4 superchunks per core
NB = L // 128             # 32 partition blocks

_CACHE = {}


def _stage_coeffs(W):
    # W: (12, 4096, 2) float -> per-stage A (diag) and B (off-diag) in f64
    Wd = np.asarray(W, dtype=np.float64)
    return Wd[:, :, 0], Wd[:, :, 1]


def _build_c1(W):
    """C1[m*128 + k, mu] = (product of stages 0..6 on block m)[mu, k]."""
    A, B = _stage_coeffs(W)
    c1 = np.empty((L, 128), dtype=np.float64)
    for m in range(NB):
        Em = np.eye(128, dtype=np.float64)
        for s in range(7):
            d = 1 << s
            a = A[s, m * 128:(m + 1) * 128]
            b = B[s, m * 128:(m + 1) * 128]
            idx = np.arange(128) ^ d
            Em = a[:, None] * Em + b[:, None] * Em[idx, :]
        # out[mu] = sum_k Em[mu, k] y[k]  ->  lhsT[k, mu] = Em[mu, k]
        c1[m * 128:(m + 1) * 128, :] = Em.T
    return c1.astype(np.float16)


def _build_c2(W):
    """C2 for residue class c: rows k = 32q + m' (q = p//32 with p = 32q+c),
    cols mu = 4m + q'.  Entry = D_{32q+c}[m', m] when q==q', else 0, where
    D_p = F_p^T and F_p is the product of stages 7..11 on the coset
    {m*128+p : m}."""
    A, B = _stage_coeffs(W)
    c2 = np.zeros((L, 128), dtype=np.float64)
    idx32 = np.arange(32)
    for c in range(32):
        for q in range(4):
            p = 32 * q + c
            Fp = np.eye(32, dtype=np.float64)
            for s in range(7, 12):
                e = (1 << s) // 128
                a = A[s, idx32 * 128 + p]
                b = B[s, idx32 * 128 + p]
                Fp = a[:, None] * Fp + b[:, None] * Fp[idx32 ^ e, :]
            # out[m] = sum_{m'} Fp[m, m'] y1[m'] -> lhsT[m', m] = Fp[m, m']
            for mp in range(32):
                for m in range(32):
                    c2[c * 128 + 32 * q + mp, 4 * m + q] = Fp[m, mp]
    return c2.astype(np.float16)


def _split_excess_waits(nc):
    """The staged walrus rejects instructions carrying more than one sync-wait
    command.  Hoist all but the last semaphore wait of each instruction onto
    fresh same-engine nops placed immediately before it (engines execute
    their stream in order, so semantics are unchanged)."""
    from concourse import mybir

    snapshots = []
    for fn in nc.m.functions:
        for blk in fn.blocks:
            snapshots.append((blk, list(blk.instructions)))
    for blk, insts in snapshots:
        changed = False
        new_list = []
        for inst in insts:
            si = inst.sync_info
            if si is not None:
                waits = list(si.on_wait)
                sem_waits = [w for w in waits if w.sync_type == "semaphore"]
                other = [w for w in waits if w.sync_type != "semaphore"]
                budget = 1 if not other else 0
                if len(sem_waits) > budget:
                    keep = sem_waits[-budget:] if budget else []
                    hoist = sem_waits[: len(sem_waits) - budget]
                    for w in hoist:
                        nop = nc.engines[inst.engine].nop(
                            hint="waitsplit", nofuse=True
                        )
                        nop.ins.sync_info = mybir.SyncInfo(
                            on_wait=[w], on_update=[]
                        )
                        new_list.append(nop.ins)
                    si.on_wait = other + keep
                    changed = True
            new_list.append(inst)
        if changed:
            blk.instructions = new_list


def _build_program(repeat: int = 1, no_compute: bool = False, no_io: bool = False, no_trans: bool = False):
    import concourse.bass as bass
    import concourse.tile as tile
    from concourse import mybir

    f32 = mybir.dt.float32
    f32r = mybir.dt.float32r
    f16 = mybir.dt.float16

    nc = bass.Bass("TRN2", num_devices=N_CORES)
    xT = nc.dram_tensor("xT", [L, BCORE], f16, kind="ExternalInput").ap()
    c1 = nc.dram_tensor("c1", [L, 128], f16, kind="ExternalInput").ap()
    c2 = nc.dram_tensor("c2", [L, 128], f16, kind="ExternalInput").ap()
    outT = nc.dram_tensor("outT", [L, BCORE], f16, kind="ExternalOutput").ap()

    xT_r = xT.rearrange("(m p) b -> p m b", p=128)    # [128, 32, BCORE]
    c1_r = c1.rearrange("(m p) k -> p m k", p=128)    # [128, 32, 128]
    c2_r = c2.rearrange("(m p) k -> p m k", p=128)
    outT_r = outT.rearrange("(t c) b -> t c b", c=32)  # [128, 32, BCORE]

    with tile.TileContext(nc) as tc:
        with (
            tc.tile_pool(name="const", bufs=1) as cpool,
            tc.tile_pool(name="xin", bufs=2) as xpool,
            tc.tile_pool(name="y1", bufs=2) as y1pool,
            tc.tile_pool(name="y1t", bufs=2) as y1tpool,
            tc.tile_pool(name="yo", bufs=3) as yopool,
            tc.tile_pool(name="ps1", bufs=2, space="PSUM") as ps1pool,
            tc.tile_pool(name="ps2", bufs=2, space="PSUM") as ps2pool,
        ):
            c1t = cpool.tile([128, NB * 128], f16, tag="c1t")
            nc.sync.dma_start(
                c1t[:].rearrange("p (m k) -> p m k", m=NB), c1_r[:]
            )
            c2t = cpool.tile([128, NB * 128], f16, tag="c2t")
            nc.sync.dma_start(
                c2t[:].rearrange("p (m k) -> p m k", m=NB), c2_r[:]
            )

            xin_shared = None
            if no_io:
                xin_shared = xpool.tile([128, NB * BC], f16, tag="xin")
                for h in range(4):
                    nc.sync.dma_start(
                        xin_shared[:, h * 8 * BC:(h + 1) * 8 * BC].rearrange(
                            "p (m b) -> p m b", m=8
                        ),
                        xT_r[:, h * 8:(h + 1) * 8, 0 * BC:(0 + 1) * BC],
                    )
            for s in range(NSC * repeat):
                s = s % NSC
                if no_io:
                    xin = xin_shared
                else:
                    xin = xpool.tile([128, NB * BC], f16, tag="xin")
                if not no_io:
                    for h in range(4):
                        nc.sync.dma_start(
                            xin[:, h * 8 * BC:(h + 1) * 8 * BC].rearrange(
                                "p (m b) -> p m b", m=8
                            ),
                            xT_r[:, h * 8:(h + 1) * 8, s * BC:(s + 1) * BC],
                        )
                if not no_compute:
                    y1 = y1pool.tile([128, NB * BC], f16, tag="y1")
                    y1t = y1tpool.tile([128, NB * BC], f16, tag="y1t")
                    for g in range(8):
                        ps = ps1pool.tile([128, 4 * BC], f32, tag="ps1")
                        for i in range(4):
                            m = 4 * g + i
                            nc.tensor.matmul(
                                ps[:, i * BC:(i + 1) * BC],
                                c1t[:, m * 128:(m + 1) * 128],
                                xin[:, m * BC:(m + 1) * BC],
                                start=True,
                                stop=True,
                            )
                        nc.any.tensor_copy(
                            y1[:].rearrange(
                                "p (k m t) -> p m k t", m=NB, t=2
                            )[:, 4 * g:4 * (g + 1), :, :],
                            ps[:].rearrange("p (m k t) -> p m k t", m=4, t=2),
                        )
                    # 32x32 partition<->free exchange: in stream (b outer, m
                    # inner), out stream (b outer, c inner)
                    if no_trans:
                        nc.vector.tensor_copy(y1t[:], y1[:])
                    else:
                        nc.vector.transpose(
                            y1t[:].bitcast(mybir.dt.float32),
                            y1[:].bitcast(mybir.dt.float32),
                        )
                for g in range(8):
                    yo = yopool.tile([128, 4 * BC], f16, tag="yo")
                    if no_compute:
                        nc.any.memset(yo[:], 0.0)
                    if not no_compute:
                        ps2 = ps2pool.tile([128, 4 * BC], f32, tag="ps2")
                        for i in range(4):
                            c = 4 * g + i
                            nc.tensor.matmul(
                                ps2[:, i * BC:(i + 1) * BC],
                                c2t[:, c * 128:(c + 1) * 128],
                                y1t[:].rearrange(
                                    "p (k c t) -> p c k t", c=NB, t=2
                                )[:, c, :, :],
                                start=True,
                                stop=True,
                            )
                        nc.any.tensor_copy(yo[:], ps2[:])
                    if not no_io:
                        nc.scalar.dma_start(
                            outT_r[:, 4 * g:4 * (g + 1), s * BC:(s + 1) * BC],
                            yo[:].rearrange("p (c b) -> p c b", c=4),
                        )
    _split_excess_waits(nc)
    return nc


def _get_program():
    if "nc" not in _CACHE:
        _CACHE["nc"] = _build_program()
    return _CACHE["nc"]


def kernel(x: np.ndarray, W: np.ndarray) -> np.ndarray:
    from concourse.bass_utils import run_bass_kernel_spmd

    c1 = _build_c1(W)
    c2 = _build_c2(W)
    xT = np.ascontiguousarray(
        np.asarray(x, dtype=np.float32).T.astype(np.float16)
    )  # [L, BATCH] fp16

    nc = _get_program()
    in_maps = []
    for core in range(N_CORES):
        sl = slice(core * BCORE, (core + 1) * BCORE)
        in_maps.append(
            {
                "xT": np.ascontiguousarray(xT[:, sl]),
                "c1": c1,
                "c2": c2,
            }
        )
    _CACHE["in_maps"] = in_maps
    res = run_bass_kernel_spmd(nc, in_maps, list(range(N_CORES)))
    out = np.empty((BATCH, L), dtype=np.float32)
    for core in range(N_CORES):
        out[core * BCORE:(core + 1) * BCORE, :] = (
            res.results[core]["outT"].T.astype(np.float32)
        )
    return out
